# revision 32
# baseline (speedup 1.0000x reference)
"""Trainium2 Bass kernel for nn_AsymmetricProjectedLinear (8 NeuronCores).

Reference computes out = x @ W_large^T with
    W_large = (A_out @ B_out) @ W_small @ (A_in @ B_in)^T    [4096, 4096]

W_large is never materialized. Factored:
    H  = B_in @ W_small^T                       [64, 1024]
    M  = H @ B_out^T                            [64, 64]
    per 256-token block: u1 = x @ A_in; t2 = u1 @ M; out = t2 @ A_out^T

Sharding: tokens (B*S = 4096) split 512/core across 8 cores; weights
replicated (a 16KB AllReduce for M costs ~50us wall on this runtime, so
every core redundantly computes M from the full W_small). Host work is
layout-only (transpose/pack/slice/dtype-cast); all FLOPs on-device.

Hard-won structure notes (from perfetto traces of prior versions):
  - The Tile scheduler orders each engine's static queue by a cost-model
    simulation that badly mispredicts DMA arrivals, and engines dispatch
    in-order, so a mis-ordered queue head-of-line blocks ready work
    (measured +20us). Every engine queue is pinned to emission order
    with sync=False dep edges: emission order here IS the schedule.
  - Sync/Scalar sequencers issue their HWDGE ring's DMAs and stall on
    ring backpressure, so the Scalar ENGINE cannot run compute until
    its ring's in-stream issues drain (~22us in). All prework drains
    therefore go on Vector; Scalar only drains stage5/t2 work that
    starts later anyway.
  - Interleaved matmul accumulation groups inside ONE psum tile compute
    garbage on HW (verified in isolation); groups must be sequential
    per tile or live in separate tiles.
  - The PE dual-pumps adjacent matmuls whose psum tiles sit in opposite
    partition halves (row- or col-offset tile_position): the second of
    the pair costs ~5ns. Used for: t2 written to BOTH halves (replaces
    an SBUF dup DMA), and stage5 pairs against the two stacked halves
    of A_out^T.
  - One M=64/N=256 chain per block for stage1 (109ns/MM, LDWEIGHTS
    hidden; N=128 chains hit a ~107ns LDW floor and double PE time).
  - The PE clock-throttles ~2x for ~5us after idle gaps; emission order
    keeps PE work dense.
  - PSUM->SBUF drains run ~95G elem/s (PSUM source caps DVE at 1x), so
    the out tiles' 2.1M elems cost ~11us split across Vector+Scalar;
    the back half is drain-bound. Out tiles are [128, 2048] (524KB
    DMAs) with enough bufs that nothing recycles.
"""

import numpy as np

import concourse.bass as bass
import concourse.mybir as mybir
import concourse.tile as tile
from concourse import bacc
from concourse.bass_utils import run_bass_kernel_spmd
from concourse.tile_rust import add_dep_helper

N_CORES = 8
Bsz, S, D = 2, 2048, 4096
TOK = Bsz * S          # 4096 tokens
T = TOK // N_CORES     # 512 tokens per core
TB = 256               # tokens per pipeline block
NBLK = T // TB         # 2 blocks
RANK = 64
DS = 1024              # d_small

F32 = mybir.dt.float32
BF16 = mybir.dt.bfloat16

_nc_cache = {}


def build():
    if "nc" in _nc_cache:
        return _nc_cache["nc"]
    nc = bacc.Bacc("TRN2", target_bir_lowering=False, debug=False,
                   num_devices=N_CORES)

    # x_p: per block, 32 d-tiles of [128, TB] packed -> [128, 32*TB]
    x_p = nc.dram_tensor("x_p", [NBLK, 128, 32 * TB], BF16, kind="ExternalInput")
    b_outT_p = nc.dram_tensor("b_outT_p", [128, 8 * RANK], BF16,
                              kind="ExternalInput")
    b_inT_p = nc.dram_tensor("b_inT_p", [128, 8 * RANK], BF16,
                             kind="ExternalInput")
    a_in_p = nc.dram_tensor("a_in_p", [128, 32 * RANK], BF16, kind="ExternalInput")
    # A_out^T stacked: parts 0-63 = cols 0:2048, parts 64-127 = cols 2048:4096
    a_out2 = nc.dram_tensor("a_out2", [128, 2048], BF16, kind="ExternalInput")
    # W_small^T packed d_in-major: chunk j = d_in rows [j*128,(j+1)*128)
    wT_p = nc.dram_tensor("wT_p", [128, 8 * DS], BF16, kind="ExternalInput")
    ident = nc.dram_tensor("ident", [RANK, RANK], BF16, kind="ExternalInput")
    out = nc.dram_tensor("out", [T, D], BF16, kind="ExternalOutput")

    # Per-engine emission-order chains (sync=False: ordering only).
    last = {}

    def chain(key, bi):
        if key in last:
            add_dep_helper(bi.ins, last[key].ins, sync=False,
                           reason="emission-order schedule")
        last[key] = bi
        return bi

    with tile.TileContext(nc) as tc:
        with (
            tc.tile_pool(name="const", bufs=1) as cpool,
            tc.tile_pool(name="xin", bufs=2) as xpool,
            tc.tile_pool(name="outp", bufs=4) as opool,
            tc.tile_pool(name="interm", bufs=2) as ipool,
            tc.tile_pool(name="ps_pre", bufs=2, space="PSUM") as ps_pre,
            tc.tile_pool(name="ps_s1", bufs=2, space="PSUM") as ps_s1,
            tc.tile_pool(name="ps_o", bufs=4, space="PSUM") as ps_o,
        ):
            # ---- input streams, byte-balanced across BOTH HWDGE rings --
            # Ring order = arrival order: weights needed by prework first,
            # then a_in, x(b0), a_out (needed only at stage5 ~24us),
            # x(b1); out DMAs chained behind.
            b_outT_s = cpool.tile([128, 8 * RANK], BF16)
            b_inT_s = cpool.tile([128, 8 * RANK], BF16)
            a_in_s = cpool.tile([128, 32 * RANK], BF16)
            a_out_s = cpool.tile([128, 2048], BF16)
            ident_s = cpool.tile([RANK, RANK], BF16)
            w_tiles = [None] * 8
            x_tiles = [[None] * 4 for _ in range(NBLK)]

            # x(b0) rides FIRST (stage1(b0) only needs a_in; it runs
            # ~13-20us and warms the PE), W + b_outT next (they gate
            # M -> t2 -> stage5 at ~24.5), a_out just before stage5
            # needs it, x(b1) last. Out DMAs chain behind on each ring.
            chain("A", nc.sync.dma_start(out=b_inT_s[:, :], in_=b_inT_p.ap()))
            chain("B", nc.scalar.dma_start(out=ident_s[:, :], in_=ident.ap()))
            chain("A", nc.sync.dma_start(out=a_in_s[:, 0:1024],
                                         in_=a_in_p.ap()[:, 0:1024]))
            chain("B", nc.scalar.dma_start(out=a_in_s[:, 1024:2048],
                                           in_=a_in_p.ap()[:, 1024:2048]))

            def x_dma(b, npiece):
                # npiece DMAs per ring; ring A carries d-tiles 0-15,
                # ring B d-tiles 16-31
                w = 16 // npiece
                for half in range(2):
                    for p in range(npiece):
                        i = half * npiece + p
                        xt = xpool.tile([128, w * TB], BF16, tag=f"x{b}_{i}")
                        eng, key = (nc.sync, "A") if half == 0 else (nc.scalar, "B")
                        c0 = (half * 16 + p * w) * TB
                        chain(key, eng.dma_start(
                            out=xt[:, :],
                            in_=x_p.ap()[b, :, c0:c0 + w * TB],
                        ))
                        x_tiles[b][i] = xt

            x_dma(0, 2)
            for j in range(8):
                wt = cpool.tile([128, DS], BF16, tag=f"w{j}")
                eng, key = (nc.sync, "A") if j % 2 == 0 else (nc.scalar, "B")
                chain(key, eng.dma_start(out=wt[:, :],
                                         in_=wT_p.ap()[:, j * DS:(j + 1) * DS]))
                w_tiles[j] = wt
            chain("B", nc.scalar.dma_start(out=b_outT_s[:, :], in_=b_outT_p.ap()))
            chain("A", nc.sync.dma_start(out=a_out_s[:, 0:1024],
                                         in_=a_out2.ap()[:, 0:1024]))
            chain("B", nc.scalar.dma_start(out=a_out_s[:, 1024:2048],
                                           in_=a_out2.ap()[:, 1024:2048]))
            x_dma(1, 2)

            # ---- prework: H -> H^T -> M ----
            # H = B_in @ W_small^T [64, 1024], accumulated over d_in
            # chunks j as they land. Two psum tiles, one group each.
            h_ps = [ps_pre.tile([RANK, 512], F32, tag="pre", name=f"h_ps{hh}")
                    for hh in range(2)]
            for j in range(8):
                for hh in range(2):
                    chain("T", nc.tensor.matmul(
                        h_ps[hh][:, :],
                        b_inT_s[:, j * RANK:(j + 1) * RANK],
                        w_tiles[j][:, hh * 512:(hh + 1) * 512],
                        start=(j == 0), stop=(j == 7),
                    ))
            h_s = cpool.tile([RANK, DS], BF16)
            chain("V", nc.vector.tensor_copy(h_s[:, 0:512], h_ps[0][:, :]))
            chain("V", nc.vector.tensor_copy(h_s[:, 512:1024], h_ps[1][:, :]))

            # ---- prework tail: H^T and M (emitted later, between
            # stage1(b0) and stage2(b0) — stage1 needs only a_in+x and
            # runs first so the PE stays dense while x(b0) lands) ----
            def prework_tail():
                hT_s = cpool.tile([128, 8 * RANK], BF16)
                for t in range(8):
                    ht_ps = ps_pre.tile([128, RANK], BF16, tag="pre")
                    chain("T", nc.tensor.transpose(
                        ht_ps[:, :], h_s[:, t * 128:(t + 1) * 128],
                        ident_s[:, :]))
                    chain("V", nc.vector.tensor_copy(
                        hT_s[:, t * RANK:(t + 1) * RANK], ht_ps[:, :]))
                # M = H @ B_out^T [r_in, r_out], accumulated over d_out
                m_s = cpool.tile([RANK, RANK], BF16)
                m_ps = ps_pre.tile([RANK, RANK], F32, tag="pre")
                for t in range(8):
                    chain("T", nc.tensor.matmul(
                        m_ps[:, :],
                        hT_s[:, t * RANK:(t + 1) * RANK],
                        b_outT_s[:, t * RANK:(t + 1) * RANK],
                        start=(t == 0), stop=(t == 7),
                    ))
                chain("V", nc.vector.tensor_copy(m_s[:, :], m_ps[:, :]))
                return m_s

            # ---- per token block ----
            u1_psb = {}

            def stage1_chunks(b, mlo, mhi):
                if b not in u1_psb:
                    u1_psb[b] = ps_s1.tile([RANK, TB], F32, tag="s1",
                                           name=f"u1_ps{b}")
                u1_ps = u1_psb[b]
                for m in range(mlo, mhi):
                    xt = x_tiles[b][m // 8]
                    col = (m % 8) * TB
                    chain("T", nc.tensor.matmul(
                        u1_ps[:, :],
                        a_in_s[:, m * RANK:(m + 1) * RANK],
                        xt[:, col:col + TB],
                        start=(m == 0), stop=(m == 31),
                    ))

            def stage1_close(b):
                u1_s = ipool.tile([RANK, TB], BF16, tag="u1")
                chain("V", nc.vector.tensor_copy(u1_s[:, :], u1_psb[b][:, :]))
                return u1_s

            def stage2(b, u1_s, m_s):
                # t2 = (u1 @ M)^T, written by the PE to BOTH partition
                # halves (col tile offset pair dual-pumps, ~5ns extra).
                # Emitted per token-half so stage5's s=0 pairs start as
                # soon as the first half is drained.
                t2_ps = ps_s1.tile([128, TB], F32, tag="s1")
                t2b = ipool.tile([128, TB], BF16, tag="t2")
                for s in range(2):
                    cols = slice(s * 128, (s + 1) * 128)
                    for ch in range(2):
                        chain("T", nc.tensor.matmul(
                            t2_ps[ch * RANK:(ch + 1) * RANK, cols],
                            m_s[:, :], u1_s[:, cols], start=True, stop=True,
                        ))
                    chain("S", nc.scalar.copy(t2b[:, cols], t2_ps[:, cols]))
                return t2b

            def stage5_pair(b, t2b, s, n, o_ts):
                # pair (s, n): po0 = tokens s-slice x out cols n*512
                # (lo half), po1 = same tokens x cols 2048+n*512
                po0 = ps_o.tile([128, 512], F32, tag="po")
                po1 = ps_o.tile([128, 512], F32, tag="po")
                chain("T", nc.tensor.matmul(
                    po0[:, :], t2b[0:RANK, s * 128:(s + 1) * 128],
                    a_out_s[0:RANK, n * 512:(n + 1) * 512],
                    start=True, stop=True,
                ))
                chain("T", nc.tensor.matmul(
                    po1[:, :], t2b[RANK:128, s * 128:(s + 1) * 128],
                    a_out_s[RANK:128, n * 512:(n + 1) * 512],
                    start=True, stop=True,
                ))
                chain("V", nc.vector.tensor_copy(
                    o_ts[0][:, n * 512:(n + 1) * 512], po0[:, :]))
                chain("S", nc.scalar.copy(
                    o_ts[1][:, n * 512:(n + 1) * 512], po1[:, :]))

            def out_dma(b, s, o_ts):
                r0 = b * TB + s * 128
                ek = [(nc.sync, "A"), (nc.scalar, "B")]
                if s == 1:
                    ek = ek[::-1]
                for cg in range(2):
                    e, key = ek[cg]
                    chain(key, e.dma_start(
                        out=out.ap()[r0:r0 + 128, cg * 2048:(cg + 1) * 2048],
                        in_=o_ts[cg][:, :]))

            # Emission = schedule. stage1(b0) first: it chases the x(b0)
            # pieces (landing ~13-17) and warms the PE for prework (H
            # chases the W chunks warm at ~18-22.5). t2(b0) is M-gated
            # (~24.5); stage5(b0) pairs are drain-gated, so stage1(b1)
            # chunks interleave into the PE idle slots as x(b1) pieces
            # land (~24-30).
            stage1_chunks(0, 0, 32)
            u1_b0 = stage1_close(0)
            m_s = prework_tail()
            t2_b0 = stage2(0, u1_b0, m_s)
            o_b0 = [[opool.tile([128, 2048], BF16, tag=f"o{s}{cg}", name=f"o0_{s}{cg}")
                     for cg in range(2)] for s in range(2)]
            pair_i = 0
            for s in range(2):
                for n in range(4):
                    stage5_pair(0, t2_b0, s, n, o_b0[s])
                    if 4 <= pair_i <= 7:
                        mlo = (pair_i - 4) * 8
                        stage1_chunks(1, mlo, mlo + 8)
                    pair_i += 1
                out_dma(0, s, o_b0[s])
            u1_b1 = stage1_close(1)
            t2_b1 = stage2(1, u1_b1, m_s)
            o_b1 = [[opool.tile([128, 2048], BF16, tag=f"o{s}{cg}", name=f"o1_{s}{cg}")
                     for cg in range(2)] for s in range(2)]
            for s in range(2):
                for n in range(4):
                    stage5_pair(1, t2_b1, s, n, o_b1[s])
                out_dma(1, s, o_b1[s])

    nc.compile()
    _nc_cache["nc"] = nc
    return nc


def _prep_in_maps(x, W_small, A_out, B_out, A_in, B_in):
    import ml_dtypes
    f = ml_dtypes.bfloat16
    x2 = np.asarray(x, dtype=f).reshape(TOK, D)
    a_in_p = np.ascontiguousarray(
        np.asarray(A_in, f).reshape(32, 128, RANK).transpose(1, 0, 2)
    ).reshape(128, 32 * RANK)
    a_outT = np.asarray(A_out, f).T            # [64, 4096]
    a_out2 = np.ascontiguousarray(
        np.concatenate([a_outT[:, :2048], a_outT[:, 2048:]], axis=0))
    b_inT_p = np.ascontiguousarray(
        np.asarray(B_in, f).T.reshape(8, 128, RANK).transpose(1, 0, 2)
    ).reshape(128, 8 * RANK)
    b_outT_p = np.ascontiguousarray(
        np.asarray(B_out, f).T.reshape(8, 128, RANK).transpose(1, 0, 2)
    ).reshape(128, 8 * RANK)
    wT_p = np.ascontiguousarray(
        np.asarray(W_small, f).T.reshape(8, 128, DS).transpose(1, 0, 2)
    ).reshape(128, 8 * DS)
    ident = np.eye(RANK, dtype=f)
    shared = {
        "b_outT_p": b_outT_p, "b_inT_p": b_inT_p, "a_in_p": a_in_p,
        "a_out2": a_out2, "wT_p": wT_p, "ident": ident,
    }
    in_maps = []
    for c in range(N_CORES):
        xs = x2[c * T:(c + 1) * T, :]            # [T, 4096]
        xp = np.ascontiguousarray(
            xs.T                                  # [4096, T]
            .reshape(32, 128, NBLK, TB)           # d-tile, p, blk, t
            .transpose(2, 1, 0, 3)                # blk, p, d-tile, t
        ).reshape(NBLK, 128, 32 * TB)
        in_maps.append({"x_p": xp, **shared})
    return in_maps


def _run(inputs, trace=False):
    nc = build()
    in_maps = _prep_in_maps(**inputs)
    res = run_bass_kernel_spmd(
        nc, in_maps, core_ids=list(range(N_CORES)), trace=trace
    )
    out = np.concatenate(
        [np.asarray(res.results[c]["out"], dtype=np.float32)
         for c in range(N_CORES)], axis=0
    ).reshape(Bsz, S, D)
    return out, res


def kernel(**inputs) -> np.ndarray:
    out, _ = _run(inputs, trace=False)
    return out


# revision 33
# speedup vs baseline: 1.0436x; 1.0436x over previous
"""Trainium2 Bass kernel for nn_AsymmetricProjectedLinear (8 NeuronCores).

Reference computes out = x @ W_large^T with
    W_large = (A_out @ B_out) @ W_small @ (A_in @ B_in)^T    [4096, 4096]

W_large is never materialized. Factored:
    H  = B_in @ W_small^T                       [64, 1024]
    M  = H @ B_out^T                            [64, 64]
    per 256-token block: u1 = x @ A_in; t2 = u1 @ M; out = t2 @ A_out^T

Sharding: tokens (B*S = 4096) split 512/core across 8 cores; weights
replicated (a 16KB AllReduce for M costs ~50us wall on this runtime, so
every core redundantly computes M from the full W_small). Host work is
layout-only (transpose/pack/slice/dtype-cast); all FLOPs on-device.

Hard-won structure notes (from perfetto traces of ~12 prior versions):
  - The Tile scheduler orders each engine's static queue by a cost-model
    simulation that badly mispredicts DMA arrivals, and engines dispatch
    in-order, so a mis-ordered queue head-of-line blocks ready work.
    Every engine queue is pinned to emission order with sync=False dep
    edges: emission order here IS the schedule.
  - Each HWDGE ring DMA costs ~1us of ring dead time on top of
    bytes/436GB/s (completion receipt), so the in-stream is packed into
    FOUR DMAs per ring: [small weights][W^T+A_out^T][x(b0)][x(b1)].
    Sync/Scalar sequencers stall on ring backpressure while in-DMAs
    queue; with 4 in-DMAs per ring both engines are free for drains by
    ~13us.
  - The PE clock-throttles ~2x when idle/sparse and needs ~4us of dense
    work to reach full rate: dummy matmuls (garbage accumulated into
    u1_ps(b0), which stage1's start=True reset wipes — also keeps them
    DCE-live) warm it up while the front weights stream.
  - Interleaved matmul accumulation groups inside ONE psum tile compute
    garbage on HW (verified in isolation); groups must be sequential
    per tile or live in separate tiles.
  - The PE dual-pumps adjacent matmuls whose psum tiles sit in opposite
    partition halves (row- or col-offset tile_position): the second of
    each pair costs ~5ns. Used for: t2 written to BOTH halves (replaces
    an SBUF dup DMA) and stage5 pairs against the two stacked halves of
    A_out^T.
  - One M=64/N=256 chain per block for stage1 (109ns/MM warm, LDWEIGHTS
    hidden; N=128 chains hit a ~107ns LDW floor and double PE time).
  - PSUM->SBUF drains run ~95G elem/s (PSUM source caps DVE at 1x), so
    the out tiles' 2.1M elems cost ~10.5us split across Vector+Scalar:
    the back half is drain-paced. stage1(b1) interleaves into the
    drain-gated stage5(b0) pair slots.
"""

import numpy as np

import concourse.bass as bass
import concourse.mybir as mybir
import concourse.tile as tile
from concourse import bacc
from concourse.bass_utils import run_bass_kernel_spmd
from concourse.tile_rust import add_dep_helper

N_CORES = 8
Bsz, S, D = 2, 2048, 4096
TOK = Bsz * S          # 4096 tokens
T = TOK // N_CORES     # 512 tokens per core
TB = 256               # tokens per pipeline block
NBLK = T // TB         # 2 blocks
RANK = 64
DS = 1024              # d_small

F32 = mybir.dt.float32
BF16 = mybir.dt.bfloat16

_nc_cache = {}


def build():
    if "nc" in _nc_cache:
        return _nc_cache["nc"]
    nc = bacc.Bacc("TRN2", target_bir_lowering=False, debug=False,
                   num_devices=N_CORES)

    # x_p: per block, 32 d-tiles of [128, TB] packed -> [128, 32*TB]
    x_p = nc.dram_tensor("x_p", [NBLK, 128, 32 * TB], BF16, kind="ExternalInput")
    # front_a = [B_in^T | A_in cols 0:1024]
    front_a = nc.dram_tensor("front_a", [128, 1536], BF16, kind="ExternalInput")
    # front_b = [ident(64 pad 128) | B_out^T | A_in cols 1024:2048]
    front_b = nc.dram_tensor("front_b", [128, 1600], BF16, kind="ExternalInput")
    # wa = [W^T chunks 0,2,4,6 | a_out2 cols 0:1024]
    # wb = [W^T chunks 1,3,5,7 | a_out2 cols 1024:2048]
    # (a_out2 = A_out^T stacked: parts 0-63 = cols 0:2048, 64-127 = rest)
    wa_p = nc.dram_tensor("wa_p", [128, 5 * DS], BF16, kind="ExternalInput")
    wb_p = nc.dram_tensor("wb_p", [128, 5 * DS], BF16, kind="ExternalInput")
    out = nc.dram_tensor("out", [T, D], BF16, kind="ExternalOutput")

    # Per-engine emission-order chains (sync=False: ordering only).
    last = {}

    def chain(key, bi):
        if key in last:
            add_dep_helper(bi.ins, last[key].ins, sync=False,
                           reason="emission-order schedule")
        last[key] = bi
        return bi

    with tile.TileContext(nc) as tc:
        with (
            tc.tile_pool(name="const", bufs=1) as cpool,
            tc.tile_pool(name="xin", bufs=2) as xpool,
            tc.tile_pool(name="outp", bufs=4) as opool,
            tc.tile_pool(name="interm", bufs=2) as ipool,
            tc.tile_pool(name="ps_pre", bufs=2, space="PSUM") as ps_pre,
            tc.tile_pool(name="ps_s1", bufs=2, space="PSUM") as ps_s1,
            tc.tile_pool(name="ps_o", bufs=4, space="PSUM") as ps_o,
        ):
            fA_s = cpool.tile([128, 1536], BF16)
            fB_s = cpool.tile([128, 1600], BF16)
            wA_s = cpool.tile([128, 5 * DS], BF16)
            wB_s = cpool.tile([128, 5 * DS], BF16)
            b_inT_s = fA_s[:, 0:512]
            ident_s = fB_s[0:RANK, 0:RANK]
            b_outT_s = fB_s[:, 64:576]

            def a_in(m):        # lhsT chunk m of A_in (d-tile m)
                if m < 16:
                    return fA_s[:, 512 + m * RANK:512 + (m + 1) * RANK]
                return fB_s[:, 576 + (m - 16) * RANK:576 + (m - 15) * RANK]

            def w_t(j):         # W^T chunk j = d_in rows [j*128,(j+1)*128)
                ws = wA_s if j % 2 == 0 else wB_s
                return ws[:, (j // 2) * DS:(j // 2 + 1) * DS]

            def a_out(half, n):  # rows half*64..., out cols n*512:(n+1)*512
                ws = wA_s if n < 2 else wB_s
                c0 = 4 * DS + (n % 2) * 512
                return ws[half * RANK:(half + 1) * RANK, c0:c0 + 512]

            x_tiles = [[None] * 2 for _ in range(NBLK)]

            # ---- 4 in-DMAs per ring; out DMAs chain behind ----
            chain("A", nc.sync.dma_start(out=fA_s[:, :], in_=front_a.ap()))
            chain("B", nc.scalar.dma_start(out=fB_s[:, :], in_=front_b.ap()))
            chain("A", nc.sync.dma_start(out=wA_s[:, :], in_=wa_p.ap()))
            chain("B", nc.scalar.dma_start(out=wB_s[:, :], in_=wb_p.ap()))

            def x_dma(b):
                for half in range(2):   # A: d-tiles 0-15, B: 16-31
                    xt = xpool.tile([128, 16 * TB], BF16, tag=f"x{half}")
                    eng, key = (nc.sync, "A") if half == 0 else (nc.scalar, "B")
                    chain(key, eng.dma_start(
                        out=xt[:, :],
                        in_=x_p.ap()[b, :, half * 16 * TB:(half + 1) * 16 * TB],
                    ))
                    x_tiles[b][half] = xt

            x_dma(0)
            x_dma(1)

            # ---- PE warmup: garbage matmuls into u1_ps(b0); stage1's
            # start=True wipes it. Runs while W streams. ----
            u1_psb = {0: ps_s1.tile([RANK, TB], F32, tag="s1", name="u1_ps0"),
                      1: None}
            for d in range(24):
                chain("T", nc.tensor.matmul(
                    u1_psb[0][:, :],
                    b_inT_s[:, (d % 8) * RANK:(d % 8 + 1) * RANK],
                    fA_s[:, 512 + (d % 4) * TB:512 + (d % 4 + 1) * TB],
                    start=(d == 0), stop=(d == 23),
                ))

            # ---- prework: H = B_in @ W^T (2 tiles, one group each) ----
            h_ps = [ps_pre.tile([RANK, 512], F32, tag="pre", name=f"h_ps{hh}")
                    for hh in range(2)]
            for j in range(8):
                for hh in range(2):
                    chain("T", nc.tensor.matmul(
                        h_ps[hh][:, :],
                        b_inT_s[:, j * RANK:(j + 1) * RANK],
                        w_t(j)[:, hh * 512:(hh + 1) * 512],
                        start=(j == 0), stop=(j == 7),
                    ))
            h_s = cpool.tile([RANK, DS], BF16)
            chain("V", nc.vector.tensor_copy(h_s[:, 0:512], h_ps[0][:, :]))
            chain("S", nc.scalar.copy(h_s[:, 512:1024], h_ps[1][:, :]))

            # ---- stage1(b0): one M=64/N=256 chain ----
            def stage1_chunks(b, mlo, mhi):
                if u1_psb[b] is None:
                    u1_psb[b] = ps_s1.tile([RANK, TB], F32, tag="s1",
                                           name=f"u1_ps{b}")
                for m in range(mlo, mhi):
                    xt = x_tiles[b][m // 16]
                    col = (m % 16) * TB
                    chain("T", nc.tensor.matmul(
                        u1_psb[b][:, :], a_in(m), xt[:, col:col + TB],
                        start=(m == 0), stop=(m == 31),
                    ))

            def stage1_close(b):
                u1_s = ipool.tile([RANK, TB], BF16, tag="u1")
                chain("V", nc.vector.tensor_copy(u1_s[:, :], u1_psb[b][:, :]))
                return u1_s

            stage1_chunks(0, 0, 32)
            u1_b0 = stage1_close(0)

            # ---- prework tail: H^T tiles then M = H @ B_out^T ----
            hT_s = cpool.tile([128, 8 * RANK], BF16)
            for t in range(8):
                ht_ps = ps_pre.tile([128, RANK], BF16, tag="pre")
                chain("T", nc.tensor.transpose(
                    ht_ps[:, :], h_s[:, t * 128:(t + 1) * 128], ident_s))
                chain("S", nc.scalar.copy(
                    hT_s[:, t * RANK:(t + 1) * RANK], ht_ps[:, :]))
            m_ps = ps_pre.tile([RANK, RANK], F32, tag="pre")
            for t in range(8):
                chain("T", nc.tensor.matmul(
                    m_ps[:, :],
                    hT_s[:, t * RANK:(t + 1) * RANK],
                    b_outT_s[:, t * RANK:(t + 1) * RANK],
                    start=(t == 0), stop=(t == 7),
                ))
            m_s = cpool.tile([RANK, RANK], BF16)
            chain("S", nc.scalar.copy(m_s[:, :], m_ps[:, :]))

            # ---- per block stage2 + stage5 ----
            def stage2(b, u1_s):
                # t2 = (u1 @ M)^T, PE-written to BOTH partition halves
                # (col tile offset pair dual-pumps); per token-half so
                # stage5's s=0 pairs start at the first half's drain.
                t2_ps = ps_s1.tile([128, TB], F32, tag="s1")
                t2b = ipool.tile([128, TB], BF16, tag="t2")
                for s in range(2):
                    cols = slice(s * 128, (s + 1) * 128)
                    for ch in range(2):
                        chain("T", nc.tensor.matmul(
                            t2_ps[ch * RANK:(ch + 1) * RANK, cols],
                            m_s[:, :], u1_s[:, cols], start=True, stop=True,
                        ))
                    chain("V", nc.vector.tensor_copy(t2b[:, cols],
                                                     t2_ps[:, cols]))
                return t2b

            def stage5_pair(b, t2b, s, n, o_ts):
                # pair (s, n): po0 = tokens s-slice x out cols n*512
                # (lo half), po1 = same tokens x cols 2048+n*512
                po0 = ps_o.tile([128, 512], F32, tag="po")
                po1 = ps_o.tile([128, 512], F32, tag="po")
                chain("T", nc.tensor.matmul(
                    po0[:, :], t2b[0:RANK, s * 128:(s + 1) * 128],
                    a_out(0, n), start=True, stop=True,
                ))
                chain("T", nc.tensor.matmul(
                    po1[:, :], t2b[RANK:128, s * 128:(s + 1) * 128],
                    a_out(1, n), start=True, stop=True,
                ))
                chain("V", nc.vector.tensor_copy(
                    o_ts[0][:, n * 512:(n + 1) * 512], po0[:, :]))
                chain("S", nc.scalar.copy(
                    o_ts[1][:, n * 512:(n + 1) * 512], po1[:, :]))

            def out_dma(b, s, o_ts):
                r0 = b * TB + s * 128
                ek = [(nc.sync, "A"), (nc.scalar, "B")]
                if s == 1:
                    ek = ek[::-1]
                for cg in range(2):
                    e, key = ek[cg]
                    chain(key, e.dma_start(
                        out=out.ap()[r0:r0 + 128, cg * 2048:(cg + 1) * 2048],
                        in_=o_ts[cg][:, :]))

            # stage5(b0) pairs are drain-paced; stage1(b1) chunks fill
            # the PE idle slots (x(b1) lands before pair 2 runs).
            t2_b0 = stage2(0, u1_b0)
            o_b0 = [[opool.tile([128, 2048], BF16, tag=f"o{s}{cg}", name=f"o0_{s}{cg}")
                     for cg in range(2)] for s in range(2)]
            pair_i = 0
            for s in range(2):
                for n in range(4):
                    stage5_pair(0, t2_b0, s, n, o_b0[s])
                    if 2 <= pair_i <= 5:
                        mlo = (pair_i - 2) * 8
                        stage1_chunks(1, mlo, mlo + 8)
                    if pair_i == 6:
                        u1_b1 = stage1_close(1)
                    if pair_i == 7:
                        t2_b1 = stage2(1, u1_b1)
                    pair_i += 1
                out_dma(0, s, o_b0[s])
            o_b1 = [[opool.tile([128, 2048], BF16, tag=f"o{s}{cg}", name=f"o1_{s}{cg}")
                     for cg in range(2)] for s in range(2)]
            for s in range(2):
                for n in range(4):
                    stage5_pair(1, t2_b1, s, n, o_b1[s])
                out_dma(1, s, o_b1[s])

    nc.compile()
    _nc_cache["nc"] = nc
    return nc


def _prep_in_maps(x, W_small, A_out, B_out, A_in, B_in):
    import ml_dtypes
    f = ml_dtypes.bfloat16
    x2 = np.asarray(x, dtype=f).reshape(TOK, D)
    a_in_p = np.ascontiguousarray(
        np.asarray(A_in, f).reshape(32, 128, RANK).transpose(1, 0, 2)
    ).reshape(128, 32 * RANK)
    a_outT = np.asarray(A_out, f).T            # [64, 4096]
    a_out2 = np.ascontiguousarray(
        np.concatenate([a_outT[:, :2048], a_outT[:, 2048:]], axis=0))
    b_inT_p = np.ascontiguousarray(
        np.asarray(B_in, f).T.reshape(8, 128, RANK).transpose(1, 0, 2)
    ).reshape(128, 8 * RANK)
    b_outT_p = np.ascontiguousarray(
        np.asarray(B_out, f).T.reshape(8, 128, RANK).transpose(1, 0, 2)
    ).reshape(128, 8 * RANK)
    wT_p = np.ascontiguousarray(
        np.asarray(W_small, f).T.reshape(8, 128, DS).transpose(1, 0, 2)
    ).reshape(128, 8 * DS)
    identp = np.zeros((128, 64), dtype=f)
    identp[:64] = np.eye(RANK, dtype=f)

    front_a = np.ascontiguousarray(
        np.concatenate([b_inT_p, a_in_p[:, :1024]], axis=1))
    front_b = np.ascontiguousarray(
        np.concatenate([identp, b_outT_p, a_in_p[:, 1024:]], axis=1))
    wa_p = np.ascontiguousarray(np.concatenate(
        [wT_p[:, 0 * DS:1 * DS], wT_p[:, 2 * DS:3 * DS],
         wT_p[:, 4 * DS:5 * DS], wT_p[:, 6 * DS:7 * DS],
         a_out2[:, :1024]], axis=1))
    wb_p = np.ascontiguousarray(np.concatenate(
        [wT_p[:, 1 * DS:2 * DS], wT_p[:, 3 * DS:4 * DS],
         wT_p[:, 5 * DS:6 * DS], wT_p[:, 7 * DS:8 * DS],
         a_out2[:, 1024:]], axis=1))

    shared = {
        "front_a": front_a, "front_b": front_b, "wa_p": wa_p, "wb_p": wb_p,
    }
    in_maps = []
    for c in range(N_CORES):
        xs = x2[c * T:(c + 1) * T, :]            # [T, 4096]
        xp = np.ascontiguousarray(
            xs.T                                  # [4096, T]
            .reshape(32, 128, NBLK, TB)           # d-tile, p, blk, t
            .transpose(2, 1, 0, 3)                # blk, p, d-tile, t
        ).reshape(NBLK, 128, 32 * TB)
        in_maps.append({"x_p": xp, **shared})
    return in_maps


def _run(inputs, trace=False):
    nc = build()
    in_maps = _prep_in_maps(**inputs)
    res = run_bass_kernel_spmd(
        nc, in_maps, core_ids=list(range(N_CORES)), trace=trace
    )
    out = np.concatenate(
        [np.asarray(res.results[c]["out"], dtype=np.float32)
         for c in range(N_CORES)], axis=0
    ).reshape(Bsz, S, D)
    return out, res


def kernel(**inputs) -> np.ndarray:
    out, _ = _run(inputs, trace=False)
    return out
